# revision 1
# baseline (speedup 1.0000x reference)
"""GateLoop (B=4, N=4096, D=1024) Trainium2 kernel over 8 NeuronCores.

Sharding: data-parallel over the 4 batch elements x 2-way tensor-parallel
split of the D=1024 recurrence channels (the complex diagonal recurrence is
independent per channel). Core c handles batch c//2, channels
[(c%2)*512 : (c%2+1)*512]. Each core computes its projections, runs the
scan over the full sequence for its 512 channels, and produces a partial
y @ wo[ch, :] of shape (1024, 4096) (transposed). The host sums the two
partials per batch and transposes back. No cross-core communication.

Scan formulation (avoids complex arithmetic + overflow): with
a_t = m_t * cis(phi_t), m_t = sigmoid(|a_t|), theta_t = arctan(ai/ar)
in (-pi/2, pi/2) (SIGNED division so the ar<0 half-plane flip folds into
the signed multiplier mt_t = m_t * sign(ar_t)). With Theta_t =
cumsum(theta) the recurrence becomes two independent REAL first-order
scans
    Zr_t = mt_t * Zr_{t-1} + kv_t * cos(Theta_t)
    Zi_t = mt_t * Zi_{t-1} + kv_t * sin(Theta_t)
and Re(S_t) = cos(Theta_t) * Zr_t + sin(Theta_t) * Zi_t, which map onto
the DVE TensorTensorScan instruction (fp32 state, |mt| < 1 so stable).

Engine balance per channel-group tile [128, 512]:
  PE:   6 proj matmul groups + out-proj (the roofline, ~413 us/core)
  Act:  squares/sign/sqrt + copies (sqrt set), sigmoid/arctan (sigmoid
        set), sin/sin/silu (silu set) -- 3 table loads per block
  DVE:  reciprocal/ratio/kv, the three scans, range reduction, bf16 tail
  Pool: r2 = sq1+sq2 and t1/t2 products (SBUF-only engine)
The out-projection of block b is emitted during block b+1 so PE never
waits on the elementwise chain at block boundaries; its PSUM result is
DMA'd directly to DRAM (no evacuation copy).
"""
import math
import os

import numpy as np
import ml_dtypes

B, N, D = 4, 4096, 1024
CH = 512            # channels per core (tensor-parallel half)
NCG = CH // 128     # 4 channel groups of 128 partitions
T = 512             # token block
NBLK = N // T
P = 128
KT = D // P         # contraction tiles
EPS = 1e-5
BF16 = ml_dtypes.bfloat16

TWO_PI = 2 * math.pi
C1 = float(np.float32(6.28125))
C2 = float(np.float32(np.float64(TWO_PI) - 6.28125))
C3 = float(np.float32(np.float64(TWO_PI) - 6.28125
                      - np.float64(np.float32(np.float64(TWO_PI) - 6.28125))))
MAGIC = float(np.float32(1.5 * 2 ** 23))
INV2PI = float(np.float32(1.0 / TWO_PI))
PI = float(np.float32(math.pi))
PIH = float(np.float32(math.pi / 2))

_NC = None
LAST_RESULT = None  # BassKernelResults of the most recent run (for profiling)


def _build():
    from contextlib import ExitStack
    from concourse import bacc
    import concourse.mybir as mybir
    import concourse.tile as tile
    from concourse.mybir import ActivationFunctionType as AF, AluOpType as OP

    fp32 = mybir.dt.float32
    bf = mybir.dt.bfloat16

    nc = bacc.Bacc(None, target_bir_lowering=False)

    xnT_d = nc.dram_tensor("xnT", [D, N], bf, kind="ExternalInput")
    wnames = ["wq", "wk", "wv", "wg", "war", "wai"]
    w_d = {n: nc.dram_tensor(n, [D, CH], bf, kind="ExternalInput") for n in wnames}
    wo_d = nc.dram_tensor("wo", [CH, D], bf, kind="ExternalInput")
    outT_d = nc.dram_tensor("outT", [D, N], bf, kind="ExternalOutput")

    xnT_t = xnT_d.rearrange("(ko p) n -> p ko n", p=P)
    outT_t = outT_d.rearrange("(mo p) n -> p mo n", p=P)

    with tile.TileContext(nc) as tc, ExitStack() as ctx:
        wpool = ctx.enter_context(tc.tile_pool(name="w", bufs=1))
        xpool = ctx.enter_context(tc.tile_pool(name="x", bufs=2))
        cpool = ctx.enter_context(tc.tile_pool(name="c", bufs=1))   # per-cg, within-block
        kpool = ctx.enter_context(tc.tile_pool(name="k", bufs=2))   # cross-block (scan carries)
        scr = ctx.enter_context(tc.tile_pool(name="s", bufs=8))     # fp32 scratch
        sbb = ctx.enter_context(tc.tile_pool(name="sb", bufs=10))   # bf16 scratch
        ypool = ctx.enter_context(tc.tile_pool(name="y", bufs=2))
        obp = ctx.enter_context(tc.tile_pool(name="o", bufs=2))
        gpool = ctx.enter_context(tc.tile_pool(name="g", bufs=2))
        pproj = ctx.enter_context(tc.tile_pool(name="pp", bufs=6, space="PSUM"))
        pout = ctx.enter_context(tc.tile_pool(name="po", bufs=2, space="PSUM"))

        # block-0 x first (first matmul needs it), then proj weights in
        # consumption order; wo only needed when block 0's out-proj runs
        # (during block 1), so it goes last.
        xb0 = xpool.tile([P, KT, T], bf, tag="xb", name="xb_0")
        nc.sync.dma_start(xb0[:], xnT_t[:, :, 0:T])
        wsb = {}
        for n in wnames:
            t_ = wpool.tile([P, KT, CH], bf, tag=f"w_{n}")
            nc.sync.dma_start(t_[:], w_d[n].rearrange("(ko p) m -> p ko m", p=P))
            wsb[n] = t_
        wosb = wpool.tile([P, CH // P, D], bf, tag="w_wo")
        nc.sync.dma_start(wosb[:], wo_d.rearrange("(ko p) m -> p ko m", p=P))

        prevTh = [None] * NCG
        prevZr = [None] * NCG
        prevZi = [None] * NCG
        prev_ys = None   # ys tiles of the previous block (out-proj deferred)
        gC = 0.0         # cross-block act-phase gate (see below)

        # Activation-table discipline: only Sqrt / {Sigmoid,Arctan} /
        # {Sin,Silu} are set-specific (Square, Sign, Copy live in every
        # set). The Tile scheduler is eager, so without extra deps it
        # interleaves phases across cgs/blocks and thrashes the 1.3us
        # table loads. Zero-valued [P,1] "gate" tiles, fed as activation
        # bias (adds 0), pin each set-specific phase after the previous
        # one: 3 loads per block.

        def emit_outproj(ys, t0):
            for mo in range(D // P):
                pso = pout.tile([P, T], fp32, tag="out")
                for cg in range(NCG):
                    nc.tensor.matmul(pso[:], wosb[:, cg, mo * P:(mo + 1) * P],
                                     ys[cg][:], start=(cg == 0), stop=(cg == NCG - 1))
                ob = obp.tile([P, T], bf, tag="ob")
                # Pool has no PSUM port; alternate the evacuation between
                # Act and DVE explicitly.
                if mo % 2 == 0:
                    nc.scalar.copy(ob[:], pso[:])
                else:
                    nc.vector.tensor_copy(ob[:], pso[:])
                nc.sync.dma_start(outT_t[:, mo, t0:t0 + T], ob[:])

        for blk in range(NBLK):
            t0 = blk * T
            if blk == 0:
                xb = xb0
            else:
                xb = xpool.tile([P, KT, T], bf, tag="xb", name=f"xb_{blk}")
                nc.sync.dma_start(xb[:], xnT_t[:, :, t0:t0 + T])

            # --- projections, cg-major; PSUM drained by phase-A ops -----
            PS = [None] * NCG
            for cg in range(NCG):
                cs = slice(cg * P, (cg + 1) * P)
                ps = {}
                for n in wnames:
                    pt = pproj.tile([P, T], fp32, tag="proj")
                    for k in range(KT):
                        nc.tensor.matmul(pt[:], wsb[n][:, k, cs], xb[:, k, :],
                                         start=(k == 0), stop=(k == KT - 1))
                    ps[n] = pt
                PS[cg] = ps

            # out-projection of the PREVIOUS block (inputs long ready, so
            # PE rolls straight through the block boundary)
            if prev_ys is not None:
                emit_outproj(prev_ys, t0 - T)

            # --- phase A (set-free Act ops + DVE drains + Sqrt) ---------
            # drains all six PSUM banks per cg + magnitude prep
            kv = [None] * NCG; qs = [None] * NCG; gb = [None] * NCG
            sgn = [None] * NCG; ratio = [None] * NCG; r = [None] * NCG
            for cg in range(NCG):
                ps = PS[cg]
                sq1 = scr.tile([P, T], fp32, tag="scr")
                nc.scalar.square(sq1[:], ps["war"][:])
                sq2 = scr.tile([P, T], fp32, tag="scr")
                nc.scalar.square(sq2[:], ps["wai"][:])
                sgn[cg] = cpool.tile([P, T], fp32, tag=f"sgn{cg}", name=f"sgn{cg}_{blk}")
                nc.scalar.sign(sgn[cg][:], ps["war"][:])
                rec = scr.tile([P, T], fp32, tag="scr")
                nc.vector.reciprocal_approx_fast(rec[:], ps["war"][:])
                rt = scr.tile([P, T], fp32, tag="scr")
                nc.vector.tensor_tensor(rt[:], ps["wai"][:], rec[:], OP.mult)
                # clamp: the HW arctan table misbehaves for huge |x|
                # (1/ar is unbounded); arctan(1e4) is within 1e-4 of pi/2
                ratio[cg] = cpool.tile([P, T], fp32, tag=f"ratio{cg}",
                                       name=f"ratio{cg}_{blk}")
                nc.vector.tensor_scalar(ratio[cg][:], rt[:], 1e4, -1e4,
                                        OP.min, OP.max)
                vs = scr.tile([P, T], fp32, tag="scr")
                nc.scalar.copy(vs[:], ps["wv"][:])
                kv[cg] = cpool.tile([P, T], bf, tag=f"kv{cg}", name=f"kv{cg}_{blk}")
                nc.vector.tensor_tensor(kv[cg][:], ps["wk"][:], vs[:], OP.mult)
                qs[cg] = cpool.tile([P, T], bf, tag=f"qs{cg}", name=f"qs{cg}_{blk}")
                nc.scalar.copy(qs[cg][:], ps["wq"][:])
                gb[cg] = cpool.tile([P, T], bf, tag=f"gb{cg}", name=f"gb{cg}_{blk}")
                nc.scalar.copy(gb[cg][:], ps["wg"][:])
                r2 = scr.tile([P, T], fp32, tag="scr")
                nc.gpsimd.tensor_tensor(r2[:], sq1[:], sq2[:], OP.add)
                r[cg] = cpool.tile([P, T], fp32, tag=f"r{cg}", name=f"r{cg}_{blk}")
                nc.scalar.activation(r[cg][:], r2[:], AF.Sqrt, bias=gC)

            # gate A: sigmoid/arctan wait for the last Sqrt
            gA = gpool.tile([P, 1], fp32, tag="gA", name=f"gA_{blk}")
            nc.vector.tensor_scalar(gA[:], r[NCG - 1][:, 0:1], 0.0, None, OP.mult)

            # --- phase B (Act set: sigmoid_and_others) ------------------
            mt = [None] * NCG; thr = [None] * NCG; Th = [None] * NCG
            ths = [None] * NCG; ms = [None] * NCG
            for cg in range(NCG):
                m = scr.tile([P, T], fp32, tag="scr")
                nc.scalar.activation(m[:], r[cg][:], AF.Sigmoid, bias=gA[:, 0:1])
                th = scr.tile([P, T], fp32, tag="scr")
                nc.scalar.activation(th[:], ratio[cg][:], AF.Arctan, bias=gA[:, 0:1])
                ms[cg], ths[cg] = m, th
                mt[cg] = cpool.tile([P, T], fp32, tag=f"mt{cg}", name=f"mt{cg}_{blk}")
                nc.vector.tensor_tensor(mt[cg][:], m[:], sgn[cg][:], OP.mult)
                Th[cg] = kpool.tile([P, T], fp32, tag=f"Th{cg}", name=f"Th{cg}_{blk}")
                init = 0.0 if blk == 0 else prevTh[cg][:, T - 1:T]
                nc.vector.tensor_tensor_scan(Th[cg][:], th[:], th[:], init,
                                             OP.add, OP.bypass)
                tmp = scr.tile([P, T], fp32, tag="scr")
                nc.vector.tensor_scalar(tmp[:], Th[cg][:], INV2PI, MAGIC,
                                        OP.mult, OP.add)
                k2 = scr.tile([P, T], fp32, tag="scr")
                nc.vector.tensor_scalar(k2[:], tmp[:], MAGIC, None, OP.subtract)
                thr[cg] = cpool.tile([P, T], fp32, tag=f"thr{cg}",
                                     name=f"thr{cg}_{blk}")
                nc.vector.cody_waite_cascade(thr[cg][:], Th[cg][:], k2[:], C1, C2, C3)
                prevTh[cg] = Th[cg]

            # gate B: sin/silu wait for the last sigmoid AND arctan
            gB = gpool.tile([P, 1], fp32, tag="gB", name=f"gB_{blk}")
            nc.vector.scalar_tensor_tensor(gB[:], ms[NCG - 1][:, 0:1], 0.0,
                                           ths[NCG - 1][:, 0:1], OP.mult, OP.mult)

            # --- phase C (Act set: silu_and_others) ---------------------
            ys = [None] * NCG
            uis = [None] * NCG; urs = [None] * NCG; sgs = [None] * NCG
            for cg in range(NCG):
                ui = sbb.tile([P, T], bf, tag="sbb", name=f"ui{cg}_{blk}")
                nc.scalar.activation(ui[:], thr[cg][:], AF.Sin, bias=gB[:, 0:1])
                thc = scr.tile([P, T], fp32, tag="scr")
                nc.vector.add_range_wrap(thc[:], thr[cg][:], PIH, PI,
                                         float(np.float32(TWO_PI)))
                ur = sbb.tile([P, T], bf, tag="sbb", name=f"ur{cg}_{blk}")
                nc.scalar.activation(ur[:], thc[:], AF.Sin, bias=gB[:, 0:1])
                sg = sbb.tile([P, T], bf, tag="sbb", name=f"sg{cg}_{blk}")
                nc.scalar.activation(sg[:], gb[cg][:], AF.Silu, bias=gB[:, 0:1])
                uis[cg], urs[cg], sgs[cg] = ui, ur, sg
                wr = sbb.tile([P, T], bf, tag="sbb", name=f"wr{cg}_{blk}")
                nc.vector.tensor_tensor(wr[:], kv[cg][:], ur[:], OP.mult)
                wi = sbb.tile([P, T], bf, tag="sbb", name=f"wi{cg}_{blk}")
                nc.vector.tensor_tensor(wi[:], kv[cg][:], ui[:], OP.mult)
                Zr = kpool.tile([P, T], bf, tag=f"Zr{cg}")
                initr = 0.0 if blk == 0 else prevZr[cg][:, T - 1:T]
                nc.vector.tensor_tensor_scan(Zr[:], mt[cg][:], wr[:], initr,
                                             OP.mult, OP.add)
                Zi = kpool.tile([P, T], bf, tag=f"Zi{cg}")
                initi = 0.0 if blk == 0 else prevZi[cg][:, T - 1:T]
                nc.vector.tensor_tensor_scan(Zi[:], mt[cg][:], wi[:], initi,
                                             OP.mult, OP.add)
                t1 = sbb.tile([P, T], bf, tag="sbb", name=f"t1{cg}_{blk}")
                nc.gpsimd.tensor_tensor(t1[:], ur[:], Zr[:], OP.mult)
                t2 = sbb.tile([P, T], bf, tag="sbb", name=f"t2{cg}_{blk}")
                nc.gpsimd.tensor_tensor(t2[:], ui[:], Zi[:], OP.mult)
                re = sbb.tile([P, T], bf, tag="sbb", name=f"re{cg}_{blk}")
                nc.vector.tensor_tensor(re[:], t1[:], t2[:], OP.add)
                y1 = sbb.tile([P, T], bf, tag="sbb", name=f"y1{cg}_{blk}")
                nc.vector.tensor_tensor(y1[:], qs[cg][:], re[:], OP.mult)
                ys[cg] = ypool.tile([P, T], bf, tag=f"y{cg}", name=f"y{cg}_{blk}")
                nc.vector.tensor_tensor(ys[cg][:], y1[:], sg[:], OP.mult)
                prevZr[cg], prevZi[cg] = Zr, Zi

            # gate C: next block's Sqrt waits for the last sin/silu
            gCt = gpool.tile([P, 1], fp32, tag="gC", name=f"gC_{blk}")
            nc.vector.scalar_tensor_tensor(gCt[:], sgs[NCG - 1][:, 0:1], 0.0,
                                           urs[NCG - 1][:, 0:1], OP.mult, OP.mult)
            gC = gCt[:, 0:1]

            prev_ys = ys

        emit_outproj(prev_ys, (NBLK - 1) * T)

    nc.finalize()
    return nc


def _get_nc():
    global _NC
    if _NC is None:
        _NC = _build()
    return _NC


def kernel(**inputs):
    global LAST_RESULT
    from concourse.bass_utils import run_bass_kernel_spmd

    x = np.asarray(inputs["x"], np.float32)
    gamma = np.asarray(inputs["gamma"], np.float32)
    wq = np.asarray(inputs["wq"], np.float32)
    wk = np.asarray(inputs["wk"], np.float32)
    wv = np.asarray(inputs["wv"], np.float32)
    wa = np.asarray(inputs["wa"], np.float32)
    wg = np.asarray(inputs["wg"], np.float32)
    wo = np.asarray(inputs["wo"], np.float32)

    inv = 1.0 / np.sqrt((x * x).sum(-1, keepdims=True) + np.float32(EPS))
    xn = (inv * x * gamma * np.float32(math.sqrt(D))).astype(np.float32)
    xnT = np.ascontiguousarray(xn.transpose(0, 2, 1)).astype(BF16)  # (B, D, N)

    in_maps = []
    for core in range(8):
        b, h = core // 2, core % 2
        ch = slice(h * CH, (h + 1) * CH)
        in_maps.append({
            "xnT": xnT[b],
            "wq": np.ascontiguousarray(wq[:, ch]).astype(BF16),
            "wk": np.ascontiguousarray(wk[:, ch]).astype(BF16),
            "wv": np.ascontiguousarray(wv[:, ch]).astype(BF16),
            "wg": np.ascontiguousarray(wg[:, ch]).astype(BF16),
            "war": np.ascontiguousarray(wa[:, h * CH:(h + 1) * CH]).astype(BF16),
            "wai": np.ascontiguousarray(wa[:, D + h * CH:D + (h + 1) * CH]).astype(BF16),
            "wo": np.ascontiguousarray(wo[ch, :]).astype(BF16),
        })

    nc = _get_nc()
    trace = bool(int(os.environ.get("GATELOOP_TRACE", "0")))
    LAST_RESULT = run_bass_kernel_spmd(
        nc, in_maps, core_ids=list(range(8)), trace=trace,
        trace_cores=list(range(8)) if trace else None,
    )
    res = LAST_RESULT.results

    out = np.empty((B, N, D), np.float32)
    for b in range(B):
        acc = (res[2 * b]["outT"].astype(np.float32)
               + res[2 * b + 1]["outT"].astype(np.float32))   # (D, N)
        out[b] = acc.T
    return out

